# revision 24
# baseline (speedup 1.0000x reference)
"""Bipartite GNN message-passing layer on 8 Trainium2 NeuronCores.

Strategy: shard target nodes across the 8 cores (6250 targets/core/
direction); partition edges by target so the scatter-mean is local to
each core; replicate the small weights.

Key implementation points (v5):
  - per-core SOURCE RENUMBERING: each core references only ~26.5K unique
    source rows (< 32768), so a per-core compacted bf16 source table lets
    every gather index fit int16 with a single index space;
  - per-edge source rows gathered HBM->SBUF in bf16 via the Q7 dma_gather
    custom op, one call per target tile over 4 SWDGE queues; pad slots
    are trailing -1 indices (skipped by the ucode: no descriptors, no
    HBM traffic) with the per-core valid count reg_load-ed from SBUF;
    the first msgs-pool rotation instead gathers pads for real (idx 0)
    so every SBUF byte is finite before any skipped slot is read
    (0 x NaN would poison the PSUM accumulation);
  - scatter matrices S are GENERATED ON CHIP (saves ~11 MB of HBM per
    core): S[e, t] = (iota[t] == tt[e]) via one DVE is_equal per
    supertile with stride-0 broadcast APs; tt (fp16 target-in-tile per
    edge slot, -1 for pads) is a 90 KB upload instead of 11 MB of fp8;
  - segment-sum on the TensorEngine: S (fp8, exact) x gathered bf16
    messages accumulated into a PSUM tile per 128 targets;
  - everything downstream in bf16: mean+residual fused DVE op, bf16 PE
    transpose, bf16 dense matmul with the bias prefilled into PSUM by a
    DVE copy; relu on ACT with row-sum accumulation; squared-sum via a
    DVE fused multiply+accum; per-direction batched LayerNorm stats;
    final scale+shift on ACT/DVE writing bf16 (host converts to f32);
  - deep software pipelining (A/B/C stages at 2-tile spacing) keeps the
    PE instruction stream gapless so it ramps to full clock; dir-u's
    apply/store supertiles are interleaved into dir-i's main loop.
"""

import os
import sys

if "/opt/trn_rl_repo" not in sys.path:
    sys.path.insert(0, "/opt/trn_rl_repo")

from contextlib import ExitStack

import ml_dtypes
import numpy as np

D = 256
NNODE = 50000
N_CORES = 8
TPC = NNODE // N_CORES  # targets per core
TILE = 128
NT = (TPC + TILE - 1) // TILE  # 49 target tiles per core
NTP = NT * TILE  # padded target rows per core (6272)
SUPER = 7  # tiles per supertile (7 x 7 = 49)
NSUP = (NT + SUPER - 1) // SUPER
GCHUNK = 8  # max gather call size in 128-edge blocks
MSGS_BUFS = 4
FULLCOVER_SUP = MSGS_BUFS  # first-rotation supertiles gather pads for real
EPS = 1e-5

F8 = ml_dtypes.float8_e4m3
BF16 = ml_dtypes.bfloat16

# test-only hooks (harness leaves these off)
_TRACE = bool(os.environ.get("BGK_TRACE"))
last_result = None

_prog_cache = {}


def _wrap_idx(idx):
    """dma_gather index layout: edge i -> [i % 16, i // 16], replicated
    across the 8 Q7-core partition groups."""
    assert len(idx) % 16 == 0
    w = idx.reshape(-1, 16).T.astype(np.int16)  # [16, n/16]
    return np.tile(w, (8, 1))  # [128, n/16]


def _call_plan(nblk):
    """Gather call layout: one or more <=GCHUNK-block calls per tile.
    Must be identical between host prep and program build."""
    plan = []  # (tile, block_lo_global, nblocks)
    off = 0
    for ti in range(len(nblk)):
        nb = int(nblk[ti])
        for c0 in range(0, nb, GCHUNK):
            plan.append((ti, off + c0, min(GCHUNK, nb - c0)))
        off += nb
    return plan


def _prep_direction(src, tgt, first_dir):
    """Host-side shard/sort/renumber/pad. Returns (meta, per-core arrays)."""
    deg = np.bincount(tgt, minlength=NNODE).astype(np.float64)
    recip_full = (1.0 / np.maximum(deg, 1.0)).astype(np.float32)

    order = np.argsort(tgt, kind="stable")
    s_all, t_all = src[order], tgt[order]
    cb = np.searchsorted(t_all, np.arange(N_CORES + 1) * TPC)

    # per-(core,tile) edge segments with per-core compact source ids
    segs = []  # [core][tile] -> (compact_src, tgt_in_tile)
    uniqs = []  # [core] -> global source ids for the compact table
    for c in range(N_CORES):
        s = s_all[cb[c] : cb[c + 1]]
        t = t_all[cb[c] : cb[c + 1]] - c * TPC
        uniq, inv = np.unique(s, return_inverse=True)
        assert len(uniq) <= 32768, (
            f"core {c}: {len(uniq)} unique sources exceed int16 gather reach"
        )
        uniqs.append(uniq)
        tb = np.searchsorted(t, np.arange(NT + 1) * TILE)
        tiles = []
        for ti in range(NT):
            ss = inv[tb[ti] : tb[ti + 1]]
            tt = t[tb[ti] : tb[ti + 1]] - ti * TILE
            tiles.append((ss, tt))
        segs.append(tiles)

    nblk = np.zeros(NT, np.int64)
    for ti in range(NT):
        mx = max(len(segs[c][ti][0]) for c in range(N_CORES))
        nblk[ti] = max(-(-mx // 128), 1)
    totblk = int(nblk.sum())
    blk_off = np.concatenate([[0], np.cumsum(nblk)])  # [NT+1]
    upad = -(-max(len(u) for u in uniqs) // 128) * 128

    # tiles that must gather pads for real (first msgs-pool rotation)
    full_tiles = (
        set(range(min(FULLCOVER_SUP * SUPER, NT))) if first_dir else set()
    )
    plan = _call_plan(nblk)

    cores = []
    for c in range(N_CORES):
        idx_cat = []
        tt_all = np.full((128, totblk), -1.0, np.float16)
        ncnt = np.zeros(NT, np.int64)  # valid idxs per tile
        for ti in range(NT):
            n = int(nblk[ti]) * 128
            ss, tt = segs[c][ti]
            fill = 0 if ti in full_tiles else -1
            idx = np.full(n, fill, np.int64)
            idx[: len(ss)] = ss
            idx_cat.append(idx)
            ncnt[ti] = n if ti in full_tiles else len(ss)
            j = np.arange(len(tt))
            tt_all[j % 128, int(blk_off[ti]) + j // 128] = tt
        # per-call valid counts; every call keeps >= 16 real descriptors
        # (ucode/interp needs at least one non-negative index per call)
        cnt = np.zeros(len(plan), np.int32)
        for k, (ti, blo, nb) in enumerate(plan):
            c0 = (blo - int(blk_off[ti])) * 128
            v = int(np.clip(ncnt[ti] - c0, 0, nb * 128))
            if v < 16:
                seg = idx_cat[ti]
                seg[c0 : c0 + 16] = np.maximum(seg[c0 : c0 + 16], 0)
                v = 16
            cnt[k] = v
        # zero-idx filler region for first-rotation buffer coverage
        idx_cat.append(np.zeros(NBMAX_PAD * 128, np.int64))
        recip = np.zeros(NTP, np.float32)
        recip[:TPC] = recip_full[c * TPC : (c + 1) * TPC]
        cores.append(
            {
                "uniq": uniqs[c],
                "idx": _wrap_idx(np.concatenate(idx_cat)),
                "cnt": cnt.reshape(1, -1),
                "tt": tt_all,
                "recip": recip.reshape(NT, 128).T.copy(),  # [128, NT]
            }
        )
    return (nblk, blk_off, totblk, upad, len(plan)), cores


NBMAX_PAD = 64  # filler idx blocks appended after the real idx stream


def _build_program(meta_u, meta_i, apply_gamma_beta):
    import concourse.bass as bass
    import concourse.tile as tile
    from concourse import bacc, mybir

    f32 = mybir.dt.float32
    f16 = mybir.dt.float16
    bf16 = mybir.dt.bfloat16
    f8 = mybir.dt.float8e4
    i16 = mybir.dt.int16
    i32 = mybir.dt.int32
    Alu = mybir.AluOpType
    Act = mybir.ActivationFunctionType

    nc = bacc.Bacc("TRN2", target_bir_lowering=False, debug=False,
                   num_devices=N_CORES, num_swdge_queues=4)

    def din(name, shape, dt):
        return nc.dram_tensor(name, shape, dt, kind="ExternalInput").ap()

    # uniform msgs buffer width across both directions
    def sup_nb(nblk, blk_off):
        return [
            int(blk_off[min((si + 1) * SUPER, NT)] - blk_off[si * SUPER])
            for si in range(NSUP)
        ]

    nbmax = max(max(sup_nb(m[0], m[1])) for m in (meta_u, meta_i))
    assert nbmax <= NBMAX_PAD

    dirs = []
    for d, (nblk, blk_off, totblk, upad, ncalls) in (
        ("u", meta_u), ("i", meta_i)
    ):
        dirs.append(
            {
                "name": d,
                "nblk": nblk,
                "blk_off": blk_off,
                "totblk": totblk,
                "plan": _call_plan(nblk),
                "src16": din(f"src16_{d}", [upad, D], bf16),
                "x": din(f"x_{d}", [NTP, D], bf16),
                "W": din(f"W_{d}", [2, 128, D], bf16),
                "bias": din(f"bias_rep_{d}", [128, D], bf16),
                "idx": din(f"idx_{d}", [128, (totblk + NBMAX_PAD) * 8], i16),
                "cnt": din(f"cnt_{d}", [1, ncalls], i32),
                "tt": din(f"tt_{d}", [128, totblk], f16),
                "recip": din(f"recip_{d}", [128, NT], f32),
                "out": nc.dram_tensor(
                    f"out_{d}", [NTP, D], bf16, kind="ExternalOutput"
                ).ap(),
            }
        )
    ident_d = din("ident", [128, 128], bf16)
    if apply_gamma_beta:
        gamma_d = din("gamma_rep", [128, D], f32)
        beta_d = din("beta_rep", [128, D], f32)

    qctr = [0]  # SWDGE queue round-robin

    with tile.TileContext(nc) as tc, ExitStack() as ctx:
        consts = ctx.enter_context(tc.tile_pool(name="consts", bufs=1))
        dmeta = ctx.enter_context(tc.tile_pool(name="dmeta", bufs=1))
        msgs_p = ctx.enter_context(tc.tile_pool(name="msgs", bufs=MSGS_BUFS))
        s_p = ctx.enter_context(tc.tile_pool(name="sp", bufs=2))
        xio_p = ctx.enter_context(tc.tile_pool(name="xio", bufs=2))
        work = ctx.enter_context(tc.tile_pool(name="work", bufs=5))
        yr_p = ctx.enter_context(tc.tile_pool(name="yrp", bufs=3))
        st_p = ctx.enter_context(tc.tile_pool(name="stp", bufs=2))
        psum_a = ctx.enter_context(tc.tile_pool(name="psa", bufs=4, space="PSUM"))
        psum_t = ctx.enter_context(tc.tile_pool(name="pst", bufs=2, space="PSUM"))
        psum_y = ctx.enter_context(tc.tile_pool(name="psy", bufs=2, space="PSUM"))
        creg = ctx.enter_context(nc.gpsimd.register(name="creg"))

        ident_t = consts.tile([128, 128], bf16)
        nc.sync.dma_start(ident_t[:], ident_d[:])
        iota_t = consts.tile([128, 128], f16)
        nc.gpsimd.iota(iota_t[:], [[1, 128]], base=0, channel_multiplier=0,
                       allow_small_or_imprecise_dtypes=True)
        if apply_gamma_beta:
            gamma_t = consts.tile([128, D], f32)
            nc.sync.dma_start(gamma_t[:], gamma_d[:])
            beta_t = consts.tile([128, D], f32)
            nc.sync.dma_start(beta_t[:], beta_d[:])

        # persistent per-direction tiles, loaded up front
        for dd in dirs:
            d = dd["name"]
            dd["W_t"] = dmeta.tile([128, 2, D], bf16, name=f"W_{d}")
            for h in range(2):
                nc.sync.dma_start(dd["W_t"][:, h, :], dd["W"][h])
            dd["bias_t"] = dmeta.tile([128, D], bf16, name=f"bias_{d}")
            nc.sync.dma_start(dd["bias_t"][:], dd["bias"][:])
            dd["recip_t"] = dmeta.tile([128, NT], f32, name=f"recip_{d}")
            nc.sync.dma_start(dd["recip_t"][:], dd["recip"][:])
            dd["idx_t"] = dmeta.tile(
                [128, (dd["totblk"] + NBMAX_PAD) * 8], i16, name=f"idx_{d}"
            )
            nc.sync.dma_start(
                dd["idx_t"][:, dd["totblk"] * 8 :],
                dd["idx"][:, dd["totblk"] * 8 :],
            )
            dd["cnt_t"] = dmeta.tile([1, len(dd["plan"])], i32, name=f"cnt_{d}")
            nc.sync.dma_start(dd["cnt_t"][:], dd["cnt"][:])
            dd["tt_t"] = dmeta.tile([128, dd["totblk"]], f16, name=f"tt_{d}")
            nc.sync.dma_start(dd["tt_t"][:], dd["tt"][:])
            dd["s1"] = dmeta.tile([128, NT], f32, name=f"s1_{d}")
            dd["s2"] = dmeta.tile([128, NT], f32, name=f"s2_{d}")
            dd["state"] = {}
            dd["callk"] = 0
            dd["yr_of"] = {}
            dd["c_done"] = {}

        def emit_A(dd, ti, msgs, b0, sb):
            d = dd["name"]
            agg = psum_a.tile([128, D], f32, tag="agg", name=f"agg_{d}_{ti}")
            nb = int(dd["nblk"][ti])
            lo = int(dd["blk_off"][ti]) - b0
            for k in range(nb):
                nc.tensor.matmul(
                    agg[:],
                    lhsT=sb[:, lo + k, :],
                    rhs=msgs[:, lo + k, :],
                    start=(k == 0),
                    stop=(k == nb - 1),
                )
            dd["state"][ti] = {"agg": agg}

        def emit_B(dd, ti, x_sup, tl):
            d = dd["name"]
            st = dd["state"][ti]
            xm = work.tile([128, D], bf16, tag="xm", name=f"xm_{d}_{ti}")
            nc.vector.scalar_tensor_tensor(
                xm[:], st["agg"][:], dd["recip_t"][:, ti : ti + 1],
                x_sup[:, tl, :], Alu.mult, Alu.add,
            )
            tr = psum_t.tile([128, 2, 128], bf16, tag="tr", name=f"tr_{d}_{ti}")
            for h in range(2):
                nc.tensor.transpose(
                    tr[:, h, :], xm[:, h * 128 : (h + 1) * 128], ident_t[:],
                )
            xmT = work.tile([128, 2, 128], bf16, tag="xmT", name=f"xmT_{d}_{ti}")
            nc.vector.tensor_copy(xmT[:], tr[:])
            st["xmT"] = xmT

        def emit_C(dd, ti):
            d = dd["name"]
            st = dd["state"].pop(ti)
            xmT = st["xmT"]
            y_ps = psum_y.tile([128, D], f32, tag="y", name=f"y_{d}_{ti}")
            # bias prefilled into PSUM by ACT; matmuls accumulate onto it
            nc.scalar.activation(y_ps[:], dd["bias_t"][:], Act.Copy)
            for h in range(2):
                nc.tensor.matmul(
                    y_ps[:], lhsT=xmT[:, h, :], rhs=dd["W_t"][:, h, :],
                    start=False, stop=(h == 1), skip_group_check=True,
                )
            sup = ti // SUPER
            yr = dd["yr_of"][sup][:, ti - sup * SUPER, :]
            nc.scalar.activation(
                yr, y_ps[:], Act.Relu, accum_out=dd["s1"][:, ti : ti + 1],
            )
            sq = work.tile([128, D], bf16, tag="sq", name=f"sq_{d}_{ti}")
            if ti % 2 == 0:
                nc.vector.scalar_tensor_tensor(
                    sq[:], yr, 1.0, yr, Alu.mult, Alu.mult,
                    accum_out=dd["s2"][:, ti : ti + 1],
                )
            else:
                nc.scalar.activation(
                    sq[:], yr, Act.Square,
                    accum_out=dd["s2"][:, ti : ti + 1],
                )
            dd["c_done"][sup] = dd["c_done"].get(sup, 0) + 1
            t1s = min((sup + 1) * SUPER, NT)
            if dd["c_done"][sup] == t1s - sup * SUPER:
                stats_apply_supertile(dd, sup)

        def main_loop(dd, fullcover):
            d = dd["name"]
            nblk, blk_off = dd["nblk"], dd["blk_off"]
            plan = dd["plan"]
            xsup_of = {}
            pend = []  # tiles awaiting B (then C)
            for si in range(NSUP):
                t0, t1 = si * SUPER, min((si + 1) * SUPER, NT)
                nts = t1 - t0
                b0, b1 = int(blk_off[t0]), int(blk_off[t1])
                nb = b1 - b0
                m = msgs_p.tile([128, nbmax, D], bf16, tag="msgs",
                                name=f"msgs_{d}_{si}")
                dd["yr_of"][si] = yr_p.tile([128, SUPER, D], bf16, tag="yr",
                                            name=f"yr_{d}_{si}")
                nc.sync.dma_start(
                    dd["idx_t"][:, b0 * 8 : b1 * 8],
                    dd["idx"][:, b0 * 8 : b1 * 8],
                )
                # scatter matrices for this supertile (DVE, off HBM)
                sb = s_p.tile([128, nbmax, 128], f8, tag="S", name=f"S_{d}_{si}")
                nc.vector.tensor_tensor(
                    sb[:, :nb, :],
                    iota_t[:].unsqueeze(1).broadcast_to([128, nb, 128]),
                    dd["tt_t"][:, b0:b1].unsqueeze(2).broadcast_to(
                        [128, nb, 128]
                    ),
                    Alu.is_equal,
                )

                x_sup = xio_p.tile([128, SUPER, D], bf16, tag="xs",
                                   name=f"xs_{d}_{si}")
                nc.sync.dma_start(
                    x_sup[:, :nts, :],
                    dd["x"][t0 * TILE : t1 * TILE].rearrange(
                        "(t p) c -> p t c", p=128
                    ),
                )
                full = fullcover and si < FULLCOVER_SUP
                for ti in range(t0, t1):
                    while (dd["callk"] < len(plan)
                           and plan[dd["callk"]][0] <= ti):
                        k = dd["callk"]
                        cti, blo, cnb = plan[k]
                        if full:
                            creg_or_const = cnb * 128
                        else:
                            nc.gpsimd.reg_load(creg, dd["cnt_t"][0:1, k : k + 1])
                            creg_or_const = creg
                        nc.gpsimd.dma_gather(
                            m[:, blo - b0 : blo - b0 + cnb, :],
                            dd["src16"],
                            dd["idx_t"][:, blo * 8 : (blo + cnb) * 8],
                            num_idxs=cnb * 128,
                            num_idxs_reg=creg_or_const,
                            elem_size=D,
                            single_packet=True,
                            queue_num=qctr[0] % 4,
                        )
                        qctr[0] += 1
                        dd["callk"] += 1
                    xsup_of[ti] = (x_sup, ti - t0)
                    emit_A(dd, ti, m, b0, sb)
                    pend.append(ti)
                    if len(pend) >= 3:
                        tb = pend[-3]
                        emit_B(dd, tb, *xsup_of.pop(tb))
                    if len(pend) >= 5:
                        emit_C(dd, pend.pop(0))
                if full and nb < nbmax:
                    # first buffer rotation: fill [nb, nbmax) with row-0
                    # gathers so no SBUF byte is ever read uninitialized
                    zb = dd["totblk"]
                    for c0 in range(nb, nbmax, GCHUNK):
                        c1 = min(c0 + GCHUNK, nbmax)
                        nc.gpsimd.dma_gather(
                            m[:, c0:c1, :],
                            dd["src16"],
                            dd["idx_t"][
                                :, (zb + c0 - nb) * 8 : (zb + c1 - nb) * 8
                            ],
                            num_idxs=(c1 - c0) * 128,
                            num_idxs_reg=(c1 - c0) * 128,
                            elem_size=D,
                            single_packet=True,
                            queue_num=qctr[0] % 4,
                        )
                        qctr[0] += 1
            # flush
            for tb in pend:
                if tb in xsup_of:
                    emit_B(dd, tb, *xsup_of.pop(tb))
            for ti in pend:
                emit_C(dd, ti)
            pend.clear()

        def stats_apply_supertile(dd, si):
            d = dd["name"]
            t0, t1 = si * SUPER, min((si + 1) * SUPER, NT)
            nts = t1 - t0
            s1s, s2s = dd["s1"][:, t0:t1], dd["s2"][:, t0:t1]
            mu = st_p.tile([128, SUPER], f32, tag="mu", name=f"mu_{d}_{si}")
            nc.vector.tensor_scalar(mu[:, :nts], s1s, 1.0 / D, None, Alu.mult)
            msq = st_p.tile([128, SUPER], f32, tag="msq", name=f"msq_{d}_{si}")
            nc.vector.tensor_tensor(msq[:, :nts], mu[:, :nts], mu[:, :nts],
                                    Alu.mult)
            var = st_p.tile([128, SUPER], f32, tag="var", name=f"var_{d}_{si}")
            nc.vector.scalar_tensor_tensor(
                var[:, :nts], s2s, 1.0 / D, msq[:, :nts], Alu.mult,
                Alu.subtract,
            )
            veps = st_p.tile([128, SUPER], f32, tag="veps", name=f"ve_{d}_{si}")
            nc.vector.tensor_scalar(veps[:, :nts], var[:, :nts], EPS, None,
                                    Alu.add)
            rv = st_p.tile([128, SUPER], f32, tag="rv", name=f"rv_{d}_{si}")
            nc.vector.reciprocal(rv[:, :nts], veps[:, :nts])
            rstd = st_p.tile([128, SUPER], f32, tag="rstd", name=f"rs_{d}_{si}")
            nc.scalar.activation(rstd[:, :nts], rv[:, :nts], Act.Sqrt)
            shift = st_p.tile([128, SUPER], f32, tag="shift",
                              name=f"sh_{d}_{si}")
            nc.vector.scalar_tensor_tensor(
                shift[:, :nts], mu[:, :nts], -1.0, rstd[:, :nts], Alu.mult,
                Alu.mult,
            )
            yr_sup = dd["yr_of"].pop(si)
            out_sup = xio_p.tile([128, SUPER, D], bf16, tag="os",
                                 name=f"os2_{d}_{si}")
            for ti in range(t0, t1):
                tl = ti - t0
                o = out_sup[:, tl, :]
                if ti % 3 == 2:
                    # spread the scale+shift across DVE and ACT
                    nc.vector.tensor_scalar(
                        o, yr_sup[:, tl, :],
                        rstd[:, tl : tl + 1], shift[:, tl : tl + 1],
                        Alu.mult, Alu.add,
                    )
                else:
                    nc.scalar.activation(
                        o, yr_sup[:, tl, :], Act.Identity,
                        bias=shift[:, tl : tl + 1],
                        scale=rstd[:, tl : tl + 1],
                    )
                if apply_gamma_beta:
                    nc.vector.tensor_tensor(o, o, gamma_t[:], Alu.mult)
                    nc.vector.tensor_tensor(o, o, beta_t[:], Alu.add)
            nc.sync.dma_start(
                dd["out"][t0 * TILE : t1 * TILE].rearrange(
                    "(t p) c -> p t c", p=128
                ),
                out_sup[:, :nts, :],
            )

        main_loop(dirs[0], fullcover=True)
        main_loop(dirs[1], fullcover=False)

    nc.compile()
    return nc


def kernel(
    user_features,
    item_features,
    user_item_edge_index,
    item_user_edge_index,
    Wu,
    bu,
    Wi,
    bi,
    gamma,
    beta,
):
    from concourse.bass_utils import run_bass_kernel_spmd

    uf = np.asarray(user_features, np.float32)
    itf = np.asarray(item_features, np.float32)
    ui = np.asarray(user_item_edge_index)
    iu = np.asarray(item_user_edge_index)
    Wu = np.asarray(Wu, np.float32)
    Wi = np.asarray(Wi, np.float32)
    bu = np.asarray(bu, np.float32)
    bi = np.asarray(bi, np.float32)
    gamma_np = np.asarray(gamma, np.float32)
    beta_np = np.asarray(beta, np.float32)

    # direction "u": targets are users, sources are items
    meta_u, cores_u = _prep_direction(
        iu[0].astype(np.int64), iu[1].astype(np.int64), first_dir=True
    )
    # direction "i": targets are items, sources are users
    meta_i, cores_i = _prep_direction(
        ui[0].astype(np.int64), ui[1].astype(np.int64), first_dir=False
    )

    apply_gb = not (np.all(gamma_np == 1.0) and np.all(beta_np == 0.0))

    key = (
        meta_u[0].tobytes(), meta_i[0].tobytes(),
        meta_u[3], meta_i[3], apply_gb,
    )
    if key not in _prog_cache:
        _prog_cache[key] = _build_program(meta_u, meta_i, apply_gb)
    nc = _prog_cache[key]

    ident = np.eye(128, dtype=BF16)
    src16_u = itf.astype(BF16)  # sources for direction u are items
    src16_i = uf.astype(BF16)

    def pad_x(x, c):
        out = np.zeros((NTP, D), BF16)
        out[:TPC] = x[c * TPC : (c + 1) * TPC]
        return out

    def src_tab(full, uniq, upad):
        tab = np.zeros((upad, D), BF16)
        tab[: len(uniq)] = full[uniq]
        return tab

    in_maps = []
    for c in range(N_CORES):
        im = {
            "src16_u": src_tab(src16_u, cores_u[c]["uniq"], meta_u[3]),
            "src16_i": src_tab(src16_i, cores_i[c]["uniq"], meta_i[3]),
            "x_u": pad_x(uf, c),
            "x_i": pad_x(itf, c),
            "W_u": Wu.astype(BF16).reshape(2, 128, D),
            "W_i": Wi.astype(BF16).reshape(2, 128, D),
            "bias_rep_u": np.tile(bu.astype(BF16)[None, :], (128, 1)),
            "bias_rep_i": np.tile(bi.astype(BF16)[None, :], (128, 1)),
            "recip_u": cores_u[c]["recip"],
            "recip_i": cores_i[c]["recip"],
            "idx_u": cores_u[c]["idx"],
            "idx_i": cores_i[c]["idx"],
            "cnt_u": cores_u[c]["cnt"],
            "cnt_i": cores_i[c]["cnt"],
            "tt_u": cores_u[c]["tt"],
            "tt_i": cores_i[c]["tt"],
            "ident": ident,
        }
        if apply_gb:
            im["gamma_rep"] = np.tile(gamma_np[None, :], (128, 1))
            im["beta_rep"] = np.tile(beta_np[None, :], (128, 1))
        in_maps.append(im)

    res = run_bass_kernel_spmd(nc, in_maps, list(range(N_CORES)), trace=_TRACE)
    global last_result
    last_result = res
    u_new = np.concatenate(
        [res.results[c]["out_u"][:TPC].astype(np.float32) for c in range(N_CORES)]
    )
    i_new = np.concatenate(
        [res.results[c]["out_i"][:TPC].astype(np.float32) for c in range(N_CORES)]
    )
    return (u_new, i_new)


# revision 25
# speedup vs baseline: 1.1132x; 1.1132x over previous
"""Bipartite GNN message-passing layer on 8 Trainium2 NeuronCores.

Strategy: shard target nodes across the 8 cores (6250 targets/core/
direction); partition edges by target so the scatter-mean is local to
each core; replicate the small weights.

Key implementation points (v5):
  - per-core SOURCE RENUMBERING: each core references only ~26.5K unique
    source rows (< 32768), so a per-core compacted bf16 source table lets
    every gather index fit int16 with a single index space;
  - per-edge source rows gathered HBM->SBUF in bf16 via the Q7 dma_gather
    custom op, one call per target tile over 4 SWDGE queues; pad slots
    are trailing -1 indices (skipped by the ucode: no descriptors, no
    HBM traffic) with the per-core valid count reg_load-ed from SBUF;
    the first msgs-pool rotation instead gathers pads for real (idx 0)
    so every SBUF byte is finite before any skipped slot is read
    (0 x NaN would poison the PSUM accumulation);
  - scatter matrices S are GENERATED ON CHIP (saves ~11 MB of HBM per
    core): S[e, t] = (iota[t] == tt[e]) via one DVE is_equal per
    supertile with stride-0 broadcast APs; tt (fp16 target-in-tile per
    edge slot, -1 for pads) is a 90 KB upload instead of 11 MB of fp8;
  - segment-sum on the TensorEngine: S (fp8, exact) x gathered bf16
    messages accumulated into a PSUM tile per 128 targets;
  - everything downstream in bf16: mean+residual fused DVE op, bf16 PE
    transpose, bf16 dense matmul with the bias prefilled into PSUM by a
    DVE copy; relu on ACT with row-sum accumulation; squared-sum via a
    DVE fused multiply+accum; per-direction batched LayerNorm stats;
    final scale+shift on ACT/DVE writing bf16 (host converts to f32);
  - deep software pipelining (A/B/C stages at 2-tile spacing) keeps the
    PE instruction stream gapless so it ramps to full clock; dir-u's
    apply/store supertiles are interleaved into dir-i's main loop.
"""

import os
import sys

if "/opt/trn_rl_repo" not in sys.path:
    sys.path.insert(0, "/opt/trn_rl_repo")

from contextlib import ExitStack

import ml_dtypes
import numpy as np

D = 256
NNODE = 50000
N_CORES = 8
TPC = NNODE // N_CORES  # targets per core
TILE = 128
NT = (TPC + TILE - 1) // TILE  # 49 target tiles per core
NTP = NT * TILE  # padded target rows per core (6272)
SUPER = 7  # tiles per supertile (7 x 7 = 49)
NSUP = (NT + SUPER - 1) // SUPER
GCHUNK = 8  # max gather call size in 128-edge blocks
MSGS_BUFS = 4
FULLCOVER_SUP = MSGS_BUFS  # first-rotation supertiles gather pads for real
EPS = 1e-5

F8 = ml_dtypes.float8_e4m3
BF16 = ml_dtypes.bfloat16

# test-only hooks (harness leaves these off)
_TRACE = bool(os.environ.get("BGK_TRACE"))
last_result = None

_prog_cache = {}


def _wrap_idx(idx):
    """dma_gather index layout: edge i -> [i % 16, i // 16], replicated
    across the 8 Q7-core partition groups."""
    assert len(idx) % 16 == 0
    w = idx.reshape(-1, 16).T.astype(np.int16)  # [16, n/16]
    return np.tile(w, (8, 1))  # [128, n/16]


def _call_plan(nblk):
    """Gather call layout: one or more <=GCHUNK-block calls per tile.
    Must be identical between host prep and program build."""
    plan = []  # (tile, block_lo_global, nblocks)
    off = 0
    for ti in range(len(nblk)):
        nb = int(nblk[ti])
        for c0 in range(0, nb, GCHUNK):
            plan.append((ti, off + c0, min(GCHUNK, nb - c0)))
        off += nb
    return plan


def _prep_direction(src, tgt, first_dir):
    """Host-side shard/sort/renumber/pad. Returns (meta, per-core arrays)."""
    deg = np.bincount(tgt, minlength=NNODE).astype(np.float64)
    recip_full = (1.0 / np.maximum(deg, 1.0)).astype(np.float32)

    order = np.argsort(tgt, kind="stable")
    s_all, t_all = src[order], tgt[order]
    cb = np.searchsorted(t_all, np.arange(N_CORES + 1) * TPC)

    # per-(core,tile) edge segments with per-core compact source ids
    segs = []  # [core][tile] -> (compact_src, tgt_in_tile)
    uniqs = []  # [core] -> global source ids for the compact table
    for c in range(N_CORES):
        s = s_all[cb[c] : cb[c + 1]]
        t = t_all[cb[c] : cb[c + 1]] - c * TPC
        uniq, inv = np.unique(s, return_inverse=True)
        assert len(uniq) <= 32768, (
            f"core {c}: {len(uniq)} unique sources exceed int16 gather reach"
        )
        uniqs.append(uniq)
        tb = np.searchsorted(t, np.arange(NT + 1) * TILE)
        tiles = []
        for ti in range(NT):
            ss = inv[tb[ti] : tb[ti + 1]]
            tt = t[tb[ti] : tb[ti + 1]] - ti * TILE
            tiles.append((ss, tt))
        segs.append(tiles)

    nblk = np.zeros(NT, np.int64)
    for ti in range(NT):
        mx = max(len(segs[c][ti][0]) for c in range(N_CORES))
        nblk[ti] = max(-(-mx // 128), 1)
    totblk = int(nblk.sum())
    blk_off = np.concatenate([[0], np.cumsum(nblk)])  # [NT+1]
    upad = -(-max(len(u) for u in uniqs) // 128) * 128

    # tiles that must gather pads for real (first msgs-pool rotation)
    full_tiles = (
        set(range(min(FULLCOVER_SUP * SUPER, NT))) if first_dir else set()
    )
    plan = _call_plan(nblk)

    cores = []
    for c in range(N_CORES):
        idx_cat = []
        tt_all = np.full((128, totblk), -1.0, np.float16)
        ncnt = np.zeros(NT, np.int64)  # valid idxs per tile
        for ti in range(NT):
            n = int(nblk[ti]) * 128
            ss, tt = segs[c][ti]
            fill = 0 if ti in full_tiles else -1
            idx = np.full(n, fill, np.int64)
            idx[: len(ss)] = ss
            idx_cat.append(idx)
            ncnt[ti] = n if ti in full_tiles else len(ss)
            j = np.arange(len(tt))
            tt_all[j % 128, int(blk_off[ti]) + j // 128] = tt
        # per-call valid counts; every call keeps >= 16 real descriptors
        # (ucode/interp needs at least one non-negative index per call)
        cnt = np.zeros(len(plan), np.int32)
        for k, (ti, blo, nb) in enumerate(plan):
            c0 = (blo - int(blk_off[ti])) * 128
            v = int(np.clip(ncnt[ti] - c0, 0, nb * 128))
            if v < 16:
                seg = idx_cat[ti]
                seg[c0 : c0 + 16] = np.maximum(seg[c0 : c0 + 16], 0)
                v = 16
            cnt[k] = v
        # zero-idx filler region for first-rotation buffer coverage
        idx_cat.append(np.zeros(NBMAX_PAD * 128, np.int64))
        recip = np.zeros(NTP, np.float32)
        recip[:TPC] = recip_full[c * TPC : (c + 1) * TPC]
        cores.append(
            {
                "uniq": uniqs[c],
                "idx": _wrap_idx(np.concatenate(idx_cat)),
                "cnt": cnt.reshape(1, -1),
                "tt": tt_all,
                "recip": recip.reshape(NT, 128).T.copy(),  # [128, NT]
            }
        )
    return (nblk, blk_off, totblk, upad, len(plan)), cores


NBMAX_PAD = 64  # filler idx blocks appended after the real idx stream


def _build_program(meta_u, meta_i, apply_gamma_beta):
    import concourse.bass as bass
    import concourse.tile as tile
    from concourse import bacc, mybir

    f32 = mybir.dt.float32
    f16 = mybir.dt.float16
    bf16 = mybir.dt.bfloat16
    f8 = mybir.dt.float8e4
    i16 = mybir.dt.int16
    i32 = mybir.dt.int32
    Alu = mybir.AluOpType
    Act = mybir.ActivationFunctionType

    nc = bacc.Bacc("TRN2", target_bir_lowering=False, debug=False,
                   num_devices=N_CORES, num_swdge_queues=4)

    def din(name, shape, dt):
        return nc.dram_tensor(name, shape, dt, kind="ExternalInput").ap()

    # uniform msgs buffer width across both directions
    def sup_nb(nblk, blk_off):
        return [
            int(blk_off[min((si + 1) * SUPER, NT)] - blk_off[si * SUPER])
            for si in range(NSUP)
        ]

    nbmax = max(max(sup_nb(m[0], m[1])) for m in (meta_u, meta_i))
    assert nbmax <= NBMAX_PAD

    dirs = []
    for d, (nblk, blk_off, totblk, upad, ncalls) in (
        ("u", meta_u), ("i", meta_i)
    ):
        dirs.append(
            {
                "name": d,
                "nblk": nblk,
                "blk_off": blk_off,
                "totblk": totblk,
                "plan": _call_plan(nblk),
                "src16": din(f"src16_{d}", [upad, D], bf16),
                "x": din(f"x_{d}", [NTP, D], bf16),
                "W": din(f"W_{d}", [2, 128, D], bf16),
                "bias": din(f"bias_rep_{d}", [128, D], bf16),
                "idx": din(f"idx_{d}", [128, (totblk + NBMAX_PAD) * 8], i16),
                "cnt": din(f"cnt_{d}", [1, ncalls], i32),
                "tt": din(f"tt_{d}", [128, totblk], f16),
                "recip": din(f"recip_{d}", [128, NT], f32),
                "out": nc.dram_tensor(
                    f"out_{d}", [NTP, D], bf16, kind="ExternalOutput"
                ).ap(),
            }
        )
    ident_d = din("ident", [128, 128], bf16)
    if apply_gamma_beta:
        gamma_d = din("gamma_rep", [128, D], f32)
        beta_d = din("beta_rep", [128, D], f32)

    qctr = [0]  # SWDGE queue round-robin

    with tile.TileContext(nc) as tc, ExitStack() as ctx:
        consts = ctx.enter_context(tc.tile_pool(name="consts", bufs=1))
        dmeta = ctx.enter_context(tc.tile_pool(name="dmeta", bufs=1))
        msgs_p = ctx.enter_context(tc.tile_pool(name="msgs", bufs=MSGS_BUFS))
        s_p = ctx.enter_context(tc.tile_pool(name="sp", bufs=2))
        xio_p = ctx.enter_context(tc.tile_pool(name="xio", bufs=2))
        work = ctx.enter_context(tc.tile_pool(name="work", bufs=5))
        yr_p = ctx.enter_context(tc.tile_pool(name="yrp", bufs=3))
        st_p = ctx.enter_context(tc.tile_pool(name="stp", bufs=2))
        psum_a = ctx.enter_context(tc.tile_pool(name="psa", bufs=4, space="PSUM"))
        psum_t = ctx.enter_context(tc.tile_pool(name="pst", bufs=2, space="PSUM"))
        psum_y = ctx.enter_context(tc.tile_pool(name="psy", bufs=2, space="PSUM"))
        creg = ctx.enter_context(nc.gpsimd.register(name="creg"))

        ident_t = consts.tile([128, 128], bf16)
        nc.sync.dma_start(ident_t[:], ident_d[:])
        iota_t = consts.tile([128, 128], f16)
        nc.gpsimd.iota(iota_t[:], [[1, 128]], base=0, channel_multiplier=0,
                       allow_small_or_imprecise_dtypes=True)
        if apply_gamma_beta:
            gamma_t = consts.tile([128, D], f32)
            nc.sync.dma_start(gamma_t[:], gamma_d[:])
            beta_t = consts.tile([128, D], f32)
            nc.sync.dma_start(beta_t[:], beta_d[:])

        # persistent per-direction tiles, loaded up front
        for dd in dirs:
            d = dd["name"]
            dd["W_t"] = dmeta.tile([128, 2, D], bf16, name=f"W_{d}")
            for h in range(2):
                nc.sync.dma_start(dd["W_t"][:, h, :], dd["W"][h])
            dd["bias_t"] = dmeta.tile([128, D], bf16, name=f"bias_{d}")
            nc.sync.dma_start(dd["bias_t"][:], dd["bias"][:])
            dd["recip_t"] = dmeta.tile([128, NT], f32, name=f"recip_{d}")
            nc.sync.dma_start(dd["recip_t"][:], dd["recip"][:])
            dd["idx_t"] = dmeta.tile(
                [128, (dd["totblk"] + NBMAX_PAD) * 8], i16, name=f"idx_{d}"
            )
            nc.sync.dma_start(
                dd["idx_t"][:, dd["totblk"] * 8 :],
                dd["idx"][:, dd["totblk"] * 8 :],
            )
            dd["cnt_t"] = dmeta.tile([1, len(dd["plan"])], i32, name=f"cnt_{d}")
            nc.sync.dma_start(dd["cnt_t"][:], dd["cnt"][:])
            dd["tt_t"] = dmeta.tile([128, dd["totblk"]], f16, name=f"tt_{d}")
            nc.sync.dma_start(dd["tt_t"][:], dd["tt"][:])
            dd["s1"] = dmeta.tile([128, NT], f32, name=f"s1_{d}")
            dd["s2"] = dmeta.tile([128, NT], f32, name=f"s2_{d}")
            dd["state"] = {}
            dd["callk"] = 0
            dd["yr_of"] = {}
            dd["c_done"] = {}

        def emit_A(dd, ti, msgs, b0, sb):
            d = dd["name"]
            agg = psum_a.tile([128, D], f32, tag="agg", name=f"agg_{d}_{ti}")
            nb = int(dd["nblk"][ti])
            lo = int(dd["blk_off"][ti]) - b0
            for k in range(nb):
                nc.tensor.matmul(
                    agg[:],
                    lhsT=sb[:, lo + k, :],
                    rhs=msgs[:, lo + k, :],
                    start=(k == 0),
                    stop=(k == nb - 1),
                )
            dd["state"][ti] = {"agg": agg}

        def emit_B(dd, ti, x_sup, tl):
            d = dd["name"]
            st = dd["state"][ti]
            xm = work.tile([128, D], bf16, tag="xm", name=f"xm_{d}_{ti}")
            nc.vector.scalar_tensor_tensor(
                xm[:], st["agg"][:], dd["recip_t"][:, ti : ti + 1],
                x_sup[:, tl, :], Alu.mult, Alu.add,
            )
            tr = psum_t.tile([128, 2, 128], bf16, tag="tr", name=f"tr_{d}_{ti}")
            for h in range(2):
                nc.tensor.transpose(
                    tr[:, h, :], xm[:, h * 128 : (h + 1) * 128], ident_t[:],
                )
            xmT = work.tile([128, 2, 128], bf16, tag="xmT", name=f"xmT_{d}_{ti}")
            nc.vector.tensor_copy(xmT[:], tr[:])
            st["xmT"] = xmT

        def emit_C(dd, ti):
            d = dd["name"]
            st = dd["state"].pop(ti)
            xmT = st["xmT"]
            y_ps = psum_y.tile([128, D], f32, tag="y", name=f"y_{d}_{ti}")
            # bias prefilled into PSUM by ACT; matmuls accumulate onto it
            nc.scalar.activation(y_ps[:], dd["bias_t"][:], Act.Copy)
            for h in range(2):
                nc.tensor.matmul(
                    y_ps[:], lhsT=xmT[:, h, :], rhs=dd["W_t"][:, h, :],
                    start=False, stop=(h == 1), skip_group_check=True,
                )
            sup = ti // SUPER
            yr = dd["yr_of"][sup][:, ti - sup * SUPER, :]
            nc.scalar.activation(
                yr, y_ps[:], Act.Relu, accum_out=dd["s1"][:, ti : ti + 1],
            )
            sq = work.tile([128, D], bf16, tag="sq", name=f"sq_{d}_{ti}")
            if ti % 2 == 0:
                nc.vector.scalar_tensor_tensor(
                    sq[:], yr, 1.0, yr, Alu.mult, Alu.mult,
                    accum_out=dd["s2"][:, ti : ti + 1],
                )
            else:
                nc.scalar.activation(
                    sq[:], yr, Act.Square,
                    accum_out=dd["s2"][:, ti : ti + 1],
                )
            dd["c_done"][sup] = dd["c_done"].get(sup, 0) + 1
            t1s = min((sup + 1) * SUPER, NT)
            if dd["c_done"][sup] == t1s - sup * SUPER:
                stats_apply_supertile(dd, sup)

        def main_loop(dd, fullcover):
            d = dd["name"]
            nblk, blk_off = dd["nblk"], dd["blk_off"]
            plan = dd["plan"]
            xsup_of = {}
            pend = []  # tiles awaiting B (then C)
            for si in range(NSUP):
                t0, t1 = si * SUPER, min((si + 1) * SUPER, NT)
                nts = t1 - t0
                b0, b1 = int(blk_off[t0]), int(blk_off[t1])
                nb = b1 - b0
                m = msgs_p.tile([128, nbmax, D], bf16, tag="msgs",
                                name=f"msgs_{d}_{si}")
                dd["yr_of"][si] = yr_p.tile([128, SUPER, D], bf16, tag="yr",
                                            name=f"yr_{d}_{si}")
                nc.sync.dma_start(
                    dd["idx_t"][:, b0 * 8 : b1 * 8],
                    dd["idx"][:, b0 * 8 : b1 * 8],
                )
                while dd["callk"] < len(plan) and plan[dd["callk"]][0] < t1:
                    k = dd["callk"]
                    ti, blo, cnb = plan[k]
                    nc.gpsimd.reg_load(creg, dd["cnt_t"][0:1, k : k + 1])
                    nc.gpsimd.dma_gather(
                        m[:, blo - b0 : blo - b0 + cnb, :],
                        dd["src16"],
                        dd["idx_t"][:, blo * 8 : (blo + cnb) * 8],
                        num_idxs=cnb * 128,
                        num_idxs_reg=creg,
                        elem_size=D,
                        single_packet=True,
                        queue_num=qctr[0] % 4,
                    )
                    qctr[0] += 1
                    dd["callk"] += 1
                if fullcover and si < FULLCOVER_SUP and nb < nbmax:
                    # first buffer rotation: fill [nb, nbmax) with row-0
                    # gathers so no SBUF byte is ever read uninitialized
                    zb = dd["totblk"]
                    for c0 in range(nb, nbmax, GCHUNK):
                        c1 = min(c0 + GCHUNK, nbmax)
                        nc.gpsimd.dma_gather(
                            m[:, c0:c1, :],
                            dd["src16"],
                            dd["idx_t"][
                                :, (zb + c0 - nb) * 8 : (zb + c1 - nb) * 8
                            ],
                            num_idxs=(c1 - c0) * 128,
                            num_idxs_reg=(c1 - c0) * 128,
                            elem_size=D,
                            single_packet=True,
                            queue_num=qctr[0] % 4,
                        )
                        qctr[0] += 1
                # on-chip scatter matrices for this supertile
                sb = s_p.tile([128, nbmax, 128], f8, tag="S", name=f"S_{d}_{si}")
                nc.vector.tensor_tensor(
                    sb[:, :nb, :],
                    iota_t[:].unsqueeze(1).broadcast_to([128, nb, 128]),
                    dd["tt_t"][:, b0:b1].unsqueeze(2).broadcast_to(
                        [128, nb, 128]
                    ),
                    Alu.is_equal,
                )

                x_sup = xio_p.tile([128, SUPER, D], bf16, tag="xs",
                                   name=f"xs_{d}_{si}")
                nc.sync.dma_start(
                    x_sup[:, :nts, :],
                    dd["x"][t0 * TILE : t1 * TILE].rearrange(
                        "(t p) c -> p t c", p=128
                    ),
                )
                for ti in range(t0, t1):
                    xsup_of[ti] = (x_sup, ti - t0)
                    emit_A(dd, ti, m, b0, sb)
                    pend.append(ti)
                    if len(pend) >= 3:
                        tb = pend[-3]
                        emit_B(dd, tb, *xsup_of.pop(tb))
                    if len(pend) >= 5:
                        emit_C(dd, pend.pop(0))
            # flush
            for tb in pend:
                if tb in xsup_of:
                    emit_B(dd, tb, *xsup_of.pop(tb))
            for ti in pend:
                emit_C(dd, ti)
            pend.clear()

        def stats_apply_supertile(dd, si):
            d = dd["name"]
            t0, t1 = si * SUPER, min((si + 1) * SUPER, NT)
            nts = t1 - t0
            s1s, s2s = dd["s1"][:, t0:t1], dd["s2"][:, t0:t1]
            mu = st_p.tile([128, SUPER], f32, tag="mu", name=f"mu_{d}_{si}")
            nc.vector.tensor_scalar(mu[:, :nts], s1s, 1.0 / D, None, Alu.mult)
            msq = st_p.tile([128, SUPER], f32, tag="msq", name=f"msq_{d}_{si}")
            nc.vector.tensor_tensor(msq[:, :nts], mu[:, :nts], mu[:, :nts],
                                    Alu.mult)
            var = st_p.tile([128, SUPER], f32, tag="var", name=f"var_{d}_{si}")
            nc.vector.scalar_tensor_tensor(
                var[:, :nts], s2s, 1.0 / D, msq[:, :nts], Alu.mult,
                Alu.subtract,
            )
            veps = st_p.tile([128, SUPER], f32, tag="veps", name=f"ve_{d}_{si}")
            nc.vector.tensor_scalar(veps[:, :nts], var[:, :nts], EPS, None,
                                    Alu.add)
            rv = st_p.tile([128, SUPER], f32, tag="rv", name=f"rv_{d}_{si}")
            nc.vector.reciprocal(rv[:, :nts], veps[:, :nts])
            rstd = st_p.tile([128, SUPER], f32, tag="rstd", name=f"rs_{d}_{si}")
            nc.scalar.activation(rstd[:, :nts], rv[:, :nts], Act.Sqrt)
            shift = st_p.tile([128, SUPER], f32, tag="shift",
                              name=f"sh_{d}_{si}")
            nc.vector.scalar_tensor_tensor(
                shift[:, :nts], mu[:, :nts], -1.0, rstd[:, :nts], Alu.mult,
                Alu.mult,
            )
            yr_sup = dd["yr_of"].pop(si)
            out_sup = xio_p.tile([128, SUPER, D], bf16, tag="os",
                                 name=f"os2_{d}_{si}")
            for ti in range(t0, t1):
                tl = ti - t0
                o = out_sup[:, tl, :]
                if ti % 3 == 2:
                    # spread the scale+shift across DVE and ACT
                    nc.vector.tensor_scalar(
                        o, yr_sup[:, tl, :],
                        rstd[:, tl : tl + 1], shift[:, tl : tl + 1],
                        Alu.mult, Alu.add,
                    )
                else:
                    nc.scalar.activation(
                        o, yr_sup[:, tl, :], Act.Identity,
                        bias=shift[:, tl : tl + 1],
                        scale=rstd[:, tl : tl + 1],
                    )
                if apply_gamma_beta:
                    nc.vector.tensor_tensor(o, o, gamma_t[:], Alu.mult)
                    nc.vector.tensor_tensor(o, o, beta_t[:], Alu.add)
            nc.sync.dma_start(
                dd["out"][t0 * TILE : t1 * TILE].rearrange(
                    "(t p) c -> p t c", p=128
                ),
                out_sup[:, :nts, :],
            )

        main_loop(dirs[0], fullcover=True)
        main_loop(dirs[1], fullcover=False)

    nc.compile()
    return nc


def kernel(
    user_features,
    item_features,
    user_item_edge_index,
    item_user_edge_index,
    Wu,
    bu,
    Wi,
    bi,
    gamma,
    beta,
):
    from concourse.bass_utils import run_bass_kernel_spmd

    uf = np.asarray(user_features, np.float32)
    itf = np.asarray(item_features, np.float32)
    ui = np.asarray(user_item_edge_index)
    iu = np.asarray(item_user_edge_index)
    Wu = np.asarray(Wu, np.float32)
    Wi = np.asarray(Wi, np.float32)
    bu = np.asarray(bu, np.float32)
    bi = np.asarray(bi, np.float32)
    gamma_np = np.asarray(gamma, np.float32)
    beta_np = np.asarray(beta, np.float32)

    # direction "u": targets are users, sources are items
    meta_u, cores_u = _prep_direction(
        iu[0].astype(np.int64), iu[1].astype(np.int64), first_dir=True
    )
    # direction "i": targets are items, sources are users
    meta_i, cores_i = _prep_direction(
        ui[0].astype(np.int64), ui[1].astype(np.int64), first_dir=False
    )

    apply_gb = not (np.all(gamma_np == 1.0) and np.all(beta_np == 0.0))

    key = (
        meta_u[0].tobytes(), meta_i[0].tobytes(),
        meta_u[3], meta_i[3], apply_gb,
    )
    if key not in _prog_cache:
        _prog_cache[key] = _build_program(meta_u, meta_i, apply_gb)
    nc = _prog_cache[key]

    ident = np.eye(128, dtype=BF16)
    src16_u = itf.astype(BF16)  # sources for direction u are items
    src16_i = uf.astype(BF16)

    def pad_x(x, c):
        out = np.zeros((NTP, D), BF16)
        out[:TPC] = x[c * TPC : (c + 1) * TPC]
        return out

    def src_tab(full, uniq, upad):
        tab = np.zeros((upad, D), BF16)
        tab[: len(uniq)] = full[uniq]
        return tab

    in_maps = []
    for c in range(N_CORES):
        im = {
            "src16_u": src_tab(src16_u, cores_u[c]["uniq"], meta_u[3]),
            "src16_i": src_tab(src16_i, cores_i[c]["uniq"], meta_i[3]),
            "x_u": pad_x(uf, c),
            "x_i": pad_x(itf, c),
            "W_u": Wu.astype(BF16).reshape(2, 128, D),
            "W_i": Wi.astype(BF16).reshape(2, 128, D),
            "bias_rep_u": np.tile(bu.astype(BF16)[None, :], (128, 1)),
            "bias_rep_i": np.tile(bi.astype(BF16)[None, :], (128, 1)),
            "recip_u": cores_u[c]["recip"],
            "recip_i": cores_i[c]["recip"],
            "idx_u": cores_u[c]["idx"],
            "idx_i": cores_i[c]["idx"],
            "cnt_u": cores_u[c]["cnt"],
            "cnt_i": cores_i[c]["cnt"],
            "tt_u": cores_u[c]["tt"],
            "tt_i": cores_i[c]["tt"],
            "ident": ident,
        }
        if apply_gb:
            im["gamma_rep"] = np.tile(gamma_np[None, :], (128, 1))
            im["beta_rep"] = np.tile(beta_np[None, :], (128, 1))
        in_maps.append(im)

    res = run_bass_kernel_spmd(nc, in_maps, list(range(N_CORES)), trace=_TRACE)
    global last_result
    last_result = res
    u_new = np.concatenate(
        [res.results[c]["out_u"][:TPC].astype(np.float32) for c in range(N_CORES)]
    )
    i_new = np.concatenate(
        [res.results[c]["out_i"][:TPC].astype(np.float32) for c in range(N_CORES)]
    )
    return (u_new, i_new)
